# revision 57
# baseline (speedup 1.0000x reference)
"""Trainium2 Bass kernel for nn_BottleneckResAtnMHSA (8 NeuronCores, SPMD).

Reference computation (per image, C=128, N=1024 spatial tokens):
  x1 = silu(bn1(w1 @ x))                      # [128, 1024]
  q/k/v = w{q,k,v} @ x1 + b{q,k,v}            # [128, 1024]
  logits = q^T k + pos^T q                    # [1024, 1024]
  att = softmax(logits, axis=-1)
  out = v @ att^T                             # [128, 1024]
  y = x + silu(bn2(w2 @ out))                 # [256, 1024]

Sharding: data-parallel over batch, 4 images per core, params replicated.

Kernel design (v3 — phase-batched, ACT-bound; 88.6us -> 70.5us):
  * Same math folding as v1: BN scales folded into conv weights host-side,
    v-bias into cv2's bias, q/k never materialized (attT = x1^T G with
    G = (Wk^T Wq) x1 + ppu precomputed constants), softmax without max
    subtraction, fp32r (tf32-class) matmuls on the logits path.
  * ACT (the bottleneck engine) runs one merged [128,1024] instruction
    per att chunk / conv tile, and the phase order batches by LUT:
    silu(a0..a3) | exp(b0..b3) | silu(c0..c3).  Auto-inserted LUT loads
    inherit the next activation's waits, so each costs its full 1283ns
    after data is ready; warm activations data-chained on early inputs
    (pcb for the Silu set, x1(a2) for the Exp set) pull both prologue
    loads into DMA-wait windows, and image 3's cv1-silu runs in tanh
    form (tanh lives in both tables; sigmoid(z)=0.5+0.5*tanh(z/2),
    combined on DVE) so nothing separates the E-load from the exps.
    Net: 3 loads, 2 hidden, vs 9 exposed in the v1 schedule.
  * x and w1 load as bf16 (halves the front DMA; host-verified end-to-end
    error ~5e-3 vs the 2e-2 budget), e and vT are bf16 (same PE rate,
    2x DVE pair-sums; logits stay fp32r so exp sees full precision).
  * Softmax denominator entirely off ACT/PE: bf16 pair-sum tree 8->1 on
    DVE, one GpSimd partition_all_reduce for the broadcast column sums
    (kills v1's all-ones PE matmul and its 2 PSUM banks).  GpSimd cannot
    touch PSUM on real HW, so all psum drains ride DVE; GpSimd takes the
    SBUF-only residual adds and the reduce.
  * PSUM tag rings sized to the pipeline ("att" [128,1024]x2, "o" x1,
    "vt" x1 = 8 banks): cv1+attention+cv2.mt1 on "att"; the v@e
    accumulators alternate "o"/"vt" so o(b+1,0) never waits the softmax
    tail outn(b); each b_main pre-emits the next image's first two att
    matmuls into whichever ring frees first, so the exp stream never
    gaps at image boundaries; cv2.mt0 takes the "o" slot in phase c.
  * DMA, one queue in strict need-order: biases | w1 | x0 | x1 | x2 |
    mqk+ppu | x3 | wvt+w2t (params deferred behind the x images that
    gate the silu chain; emission is interleaved so every copy still
    precedes its consumers), outputs per-128-channel half as soon as
    each residual add lands.
"""

import numpy as np

N_CORES = 8
B_PER_CORE = 4
C = 128
CIN = 256
N = 1024
EPS = 1e-5

PCB_W = 2 * C                        # w1t (2 k-chunks), bf16
PC0_W = 4                            # t1 | t1h | b2a | b2b (f32 biases)
PCA_W = C + N                        # mqk | ppu
PCB2_W = 2 * C + CIN                 # wvt,wvt | w2t

_CACHE = {}


def _build_program(native_silu=True, use_f32r=True):
    import concourse.bacc as bacc
    import concourse.bass_isa as bass_isa
    import concourse.mybir as mybir
    import concourse.tile as tile

    f32 = mybir.dt.float32
    bf16 = mybir.dt.bfloat16
    AF = mybir.ActivationFunctionType
    ALU = mybir.AluOpType

    nc = bacc.Bacc("TRN2", target_bir_lowering=False, debug=False,
                   num_devices=N_CORES)

    xs = nc.dram_tensor("xs", [B_PER_CORE, CIN, N], bf16, kind="ExternalInput").ap()
    pcb = nc.dram_tensor("pcb", [C, PCB_W], bf16, kind="ExternalInput").ap()
    pc0 = nc.dram_tensor("pc0", [C, PC0_W], f32, kind="ExternalInput").ap()
    pca = nc.dram_tensor("pca", [C, PCA_W], f32, kind="ExternalInput").ap()
    pc2 = nc.dram_tensor("pc2", [C, PCB2_W], f32, kind="ExternalInput").ap()
    ys = nc.dram_tensor("ys", [B_PER_CORE, CIN, N], f32, kind="ExternalOutput").ap()

    HALF = [slice(0, 512), slice(512, 1024)]

    with tile.TileContext(nc) as tc:
        with (
            tc.tile_pool(name="consts", bufs=1) as consts,
            tc.tile_pool(name="act", bufs=1) as act,
            tc.tile_pool(name="psum", bufs=2, space="PSUM") as psum,
        ):
            pcb_sb = consts.tile([128, PCB_W], bf16, tag="pcb")
            pc0_sb = consts.tile([128, PC0_W], f32, tag="pc0")
            pca_sb = consts.tile([128, PCA_W], f32, tag="pca")
            pc2_sb = consts.tile([128, PCB2_W], f32, tag="pc2")
            w1t_sb = pcb_sb.rearrange("p (k m) -> p k m", k=2)
            t1c_sb = pc0_sb[:, 0:1]
            t1h_sb = pc0_sb[:, 1:2]
            b2c_sb = pc0_sb[:, 2:4]
            mqk_sb = pca_sb[:, 0:C]
            ppu_sb = pca_sb[:, C:]
            wvt_sb = pc2_sb[:, 0:2 * C]
            w2t_sb = pc2_sb[:, 2 * C:]

            fr = (lambda ap: ap.bitcast(mybir.dt.float32r)) if use_f32r \
                else (lambda ap: ap)
            frw = fr  # producer-side marker: fp32r-matmul operands are
            # written as fp32r (same 4-byte layout)

            def mm(out, lhsT, rhs, **kw):
                nc.tensor.matmul(out, fr(lhsT), fr(rhs), **kw)

            def mmb(out, lhsT, rhs, **kw):
                nc.tensor.matmul(out, lhsT, rhs, **kw)

            def silu(out_sb, ps, bias_col, round_r=False):
                """out = silu(ps + bias); ps is a [128,1024] psum."""
                w = frw if round_r else (lambda a: a)
                if native_silu:
                    nc.scalar.activation(w(out_sb), ps, AF.Silu,
                                         bias=bias_col)
                else:
                    # CoreSim has no Silu LUT: silu(z) = z * sigmoid(z)
                    nc.scalar.activation(out_sb, ps, AF.Sigmoid,
                                         bias=bias_col)
                    nc.vector.scalar_tensor_tensor(
                        w(out_sb), ps, bias_col, out_sb,
                        op0=ALU.add, op1=ALU.mult)

            x_sbs, x1_sbs, g_sbs, vt_sbs, outn_sbs = [], [], [], [], []

            def load_x(b):
                x_sb = act.tile([128, 2, N], bf16, tag="x", bufs=4, name=f"x{b}")
                xr = xs[b].rearrange("(k p) n -> p k n", p=128)
                # strict need-order: biases+w1 | x0 | x1 | mqk+ppu |
                # x2 | x3 | wvt+w2t -- the G-path params overlap the x2
                # transfer, vT/cv2 params ride behind everything x
                if b == 0:
                    nc.sync.dma_start(pc0_sb, pc0)
                    nc.sync.dma_start(pcb_sb, pcb)
                nc.sync.dma_start(x_sb, xr)
                if b == 2:
                    # mqk/ppu ride behind x2 (G(a0) isn't ready sooner);
                    # emission stays ahead of every consumer
                    nc.sync.dma_start(frw(pca_sb), fr(pca))
                if b == 3:
                    nc.sync.dma_start(frw(pc2_sb), fr(pc2))
                x_sbs.append(x_sb)

            warm_sb = consts.tile([128, 1], f32, tag="warm")

            def cv1(b):
                # auto-inserted LUT loads inherit the next activation's
                # waits, so each swap costs 1283ns AFTER its data is ready
                # -- warm activations chained on early-available inputs
                # pull the loads into otherwise-idle ACT windows.
                if b == 0 and native_silu:
                    # S-load rides the pcb DMA (~2.8us) instead of the
                    # cv1(a0) matmul chain (~7.4us)
                    nc.scalar.activation(warm_sb, pcb_sb[:, 0:2].bitcast(f32),
                                         AF.Silu, scale=0.0)
                tanh_form = b == B_PER_CORE - 1 and native_silu
                last = b == B_PER_CORE - 1 and native_silu
                ps = psum.tile([128, N], f32, tag="o" if last else "att",
                               bufs=1 if last else None, name=f"psx1_{b}")
                for h in HALF:
                    for k in range(2):
                        mmb(ps[:, h], w1t_sb[:, k, :], x_sbs[b][:, k, h],
                            start=(k == 0), stop=(k == 1))
                x1_sb = act.tile([128, N], f32, tag="x1", bufs=4, name=f"x1_{b}")
                if tanh_form:
                    # last image: silu via tanh, which lives in BOTH the
                    # silu and exp tables; a warm exp data-chained behind
                    # silu(a2) swaps the table during the x3 DMA wait, so
                    # neither tanh(a3) nor the b-phase exps pay a LUT load
                    # on the critical ACT path.
                    # sigmoid(z) = 0.5 + 0.5*tanh(z/2); silu = z*sigmoid.
                    nc.scalar.activation(warm_sb, x1_sbs[b - 1][:, 0:1],
                                         AF.Exp, scale=0.0)
                    u_sb = act.tile([128, N], f32, tag="u", bufs=2,
                                    name=f"u{b}")
                    nc.scalar.activation(u_sb, ps, AF.Tanh, scale=0.5,
                                         bias=t1h_sb)
                    nc.vector.tensor_scalar(u_sb, u_sb, 0.5, 0.5,
                                            op0=ALU.mult, op1=ALU.add)
                    nc.vector.scalar_tensor_tensor(
                        frw(x1_sb), ps, t1c_sb, u_sb,
                        op0=ALU.add, op1=ALU.mult)
                else:
                    silu(x1_sb, ps, t1c_sb, round_r=use_f32r)
                x1_sbs.append(x1_sb)

            def g_part(b):
                x1_sb = x1_sbs[b]

                # G = (Wk^T Wq) @ x1 + (Wq^T pos + (Wk^T bq) 1^T); the
                # attention logits are then attT = x1^T G (K=128, single
                # layer) -- q and k are never materialized, and all
                # i-only bias terms cancel in the softmax.
                ps = psum.tile([128, N], f32, tag="vt", bufs=1,
                               name=f"psg_{b}")
                for h in HALF:
                    mm(ps[:, h], mqk_sb, x1_sb[:, h], start=True, stop=True)
                g_sb = act.tile([128, N], f32, tag="g", bufs=4, name=f"g{b}")
                nc.vector.tensor_add(frw(g_sb), ps, ppu_sb)
                g_sbs.append(g_sb)

            def vt_part(b):
                x1_sb = x1_sbs[b]
                # vT in two 4-token-tile chunks; each matmul writes
                # [128, 256] (wvt duplicated along free dim keeps the
                # fp32r moving dim at 256 = full rate; half unused);
                # GpSimd strided-copies out the first halves as bf16.
                vt_sb = act.tile([128, N], bf16, tag="vt", bufs=4, name=f"vt{b}")
                for c2 in range(2):
                    ps = psum.tile([128, 4, 2, C], f32, tag="vt", bufs=1,
                                   name=f"psvt_{b}_{c2}")
                    for it in range(4):
                        t0 = (c2 * 4 + it) * 128
                        mm(ps[:, it], x1_sb[:, t0:t0 + 128], wvt_sb,
                           start=True, stop=True)
                    # GPSIMD cannot read PSUM on real HW; drain on DVE
                    nc.vector.tensor_copy(
                        vt_sb[:, c2 * 512:(c2 + 1) * 512].rearrange(
                            "p (i c) -> p i c", i=4),
                        ps[:, :, 0, :])
                vt_sbs.append(vt_sb)


            pre_atts = {}
            b_state = {}

            def emit_att(b, jt, tag="att"):
                # attT[j,i] = sum_c x1[c,j] G[c,i]: K=128, single layer
                sl = slice(jt * 128, (jt + 1) * 128)
                ps = psum.tile([128, N], f32, tag=tag,
                               bufs=None if tag == "att" else 1,
                               name=f"psatt_{b}_{jt}")
                for h in HALF:
                    mm(ps[:, h], x1_sbs[b][:, sl], g_sbs[b][:, h],
                       start=True, stop=True)
                return ps

            def b_main(b, pre_next=False):
                # attention, software-pipelined (lag-1): PE runs att(jt+2)
                # and o(jt-1) while ACT exps att(jt); DVE pair-sums the
                # bf16 e tiles 8->1 (finished in b_tail).  pre_next emits
                # the next image's first two att matmuls ahead of o(7) so
                # its exps start with zero ACT gap.
                vt_sb = vt_sbs[b]
                # alternate the o-accumulator between the "o" and "vt"
                # rings: each frees ~outn-time later, and alternating
                # means o(b+1,0) never waits on outn(b) (the softmax tail)
                ps_o = psum.tile([128, N], f32, tag="o" if b % 2 == 0 else "vt",
                                 bufs=1, name=f"pso_{b}")
                e_sb = act.tile([128, 8, N], bf16, tag="e", bufs=2, name=f"e{b}")
                ep_sb = act.tile([128, 4, N], bf16, tag="ep", bufs=2,
                                 name=f"ep{b}")
                b_state[b] = (ps_o, e_sb, ep_sb)

                def emit_o(jt):
                    sl = slice(jt * 128, (jt + 1) * 128)
                    for h in HALF:
                        # numerator: out[c, i] += sum_j v[c, j] e[j, i]
                        mmb(ps_o[:, h], vt_sb[:, sl], e_sb[:, jt, h],
                            start=(jt == 0), stop=(jt == 7),
                            skip_group_check=True)

                atts = {jt: pre_atts.pop((b, jt), None) or emit_att(b, jt)
                        for jt in (0, 1)}
                for jt in range(8):
                    cur = atts.pop(jt)
                    nc.scalar.activation(e_sb[:, jt, :], cur, AF.Exp)
                    if jt + 2 <= 7:
                        atts[jt + 2] = emit_att(b, jt + 2)
                    elif jt == 7 and pre_next:
                        # jt=0 of the next image rides whichever of the
                        # "o"/"vt" rings frees first (slotted between the
                        # alternating o-accumulators) so its exp starts
                        # the moment exp(b,7) retires; the att ring
                        # itself only frees at exp(b,6)
                        tag0 = {1: "att", 2: "o", 3: "vt"}[b + 1]
                        pre_atts[(b + 1, 0)] = emit_att(b + 1, 0, tag=tag0)
                        pre_atts[(b + 1, 1)] = emit_att(b + 1, 1)
                    if jt >= 1:
                        emit_o(jt - 1)
                    if jt % 2 == 1:
                        p = jt // 2
                        nc.vector.tensor_add(ep_sb[:, p, :],
                                             e_sb[:, jt - 1, :],
                                             e_sb[:, jt, :])
                emit_o(7)

            def b_tail(b):
                ps_o, e_sb, ep_sb = b_state.pop(b)
                eq_sb = act.tile([128, 2, N], bf16, tag="eq", bufs=2,
                                 name=f"eq{b}")
                et_sb = act.tile([128, N], bf16, tag="et", bufs=2, name=f"et{b}")
                sb_sb = act.tile([128, N], f32, tag="sb", bufs=2, name=f"sb{b}")
                rc_sb = act.tile([128, N], f32, tag="rc", bufs=2, name=f"rc{b}")
                nc.vector.tensor_add(eq_sb[:, 0, :],
                                     ep_sb[:, 0, :], ep_sb[:, 1, :])
                nc.vector.tensor_add(eq_sb[:, 1, :],
                                     ep_sb[:, 2, :], ep_sb[:, 3, :])
                nc.vector.tensor_add(et_sb, eq_sb[:, 0, :], eq_sb[:, 1, :])
                nc.gpsimd.partition_all_reduce(
                    sb_sb, et_sb, channels=128,
                    reduce_op=bass_isa.ReduceOp.add)
                nc.vector.reciprocal(rc_sb, sb_sb)
                outn_sb = act.tile([128, N], f32, tag="outn", bufs=4,
                                   name=f"outn{b}")
                nc.vector.tensor_mul(frw(outn_sb), ps_o, rc_sb)
                outn_sbs.append(outn_sb)

            def phase_c(b):
                res_sb = act.tile([128, 2, N], f32, tag="res", bufs=4,
                                  name=f"res{b}")
                yr = ys[b].rearrange("(k p) n -> p k n", p=128)
                for mt in range(2):
                    sl = slice(mt * 128, (mt + 1) * 128)
                    # cv2 psums ride the att ring: its slots free in
                    # b3-exp order, which keeps the greedy per-engine
                    # scheduler from hoisting c-phase silus into the exp
                    # stream (each hoist costs two 1283ns LUT swaps).
                    # c0.mt0 takes the "o" slot (free at outn(b2)) so the
                    # first silu runs right after the table load.
                    first = b == 0 and mt == 0
                    ps = psum.tile([128, N], f32, tag="o" if first else "att",
                                   bufs=1 if first else None,
                                   name=f"psy_{b}_{mt}")
                    for h in HALF:
                        mm(ps[:, h], w2t_sb[:, sl], outn_sbs[b][:, h],
                           start=True, stop=True)
                    silu(res_sb[:, mt, :], ps, b2c_sb[:, mt:mt + 1])
                    # split the residual adds across DVE and GpSimd so
                    # neither serializes the output stream
                    eng = nc.vector if (mt == 0 or b == B_PER_CORE - 1) \
                        else nc.gpsimd
                    eng.tensor_add(res_sb[:, mt, :], res_sb[:, mt, :],
                                   x_sbs[b][:, mt, :])
                    nc.sync.dma_start(yr[:, mt, :], res_sb[:, mt, :])

            # emission keeps every DMA ahead of its consumers while the
            # sync queue stays in need-order: biases+w1, x0, x1, x2,
            # mqk+ppu, x3, wvt+w2t
            load_x(0); cv1(0)
            load_x(1); cv1(1)
            load_x(2)
            g_part(0); g_part(1)
            cv1(2)
            load_x(3)
            g_part(2)
            vt_part(0); vt_part(1)
            cv1(3)
            vt_part(2)
            g_part(3); vt_part(3)
            for b in range(B_PER_CORE):
                b_main(b, pre_next=(b < B_PER_CORE - 1))
                b_tail(b)
            for b in range(B_PER_CORE):
                phase_c(b)

    nc.compile()
    return nc


def _prepare_params(w1, bn1_g, bn1_b, bn1_m, bn1_v, wq, bq, wk, bk, wv, bv,
                    rel_h, rel_w, w2, bn2_g, bn2_b, bn2_m, bn2_v):
    import ml_dtypes

    f64 = np.float64
    s1 = bn1_g.astype(f64) / np.sqrt(bn1_v.astype(f64) + EPS)
    w1p = w1.astype(f64) * s1[:, None]
    t1 = bn1_b.astype(f64) - bn1_m.astype(f64) * s1
    s2 = bn2_g.astype(f64) / np.sqrt(bn2_v.astype(f64) + EPS)
    w2p = w2.astype(f64) * s2[:, None]
    t2 = bn2_b.astype(f64) - bn2_m.astype(f64) * s2
    b2 = t2 + w2p @ bv.astype(f64)
    posv = (rel_h.astype(f64) + rel_w.astype(f64)).reshape(C, N)
    f32 = np.float32
    mqk_m = wq.astype(f64).T @ wk.astype(f64)
    ppu_m = wq.astype(f64).T @ posv + (wk.astype(f64).T @ bq.astype(f64))[:, None]

    # pcb = w1t (2 k-chunks) as bf16 -> [128, 256]
    w1tp = np.ascontiguousarray(w1p.T).reshape(2, 128, C).transpose(1, 0, 2)
    pcb = np.ascontiguousarray(w1tp.reshape(128, 2 * C),
                               dtype=ml_dtypes.bfloat16)

    wvtp = np.ascontiguousarray(wv.T)
    w2tp = np.ascontiguousarray(w2p.T)
    pc0 = np.stack([t1, 0.5 * t1, b2[:128], b2[128:]], axis=1)
    pca = np.concatenate([mqk_m, ppu_m], axis=1)
    pc2 = np.concatenate([wvtp, wvtp, w2tp], axis=1)

    return {
        "pcb": pcb,
        "pc0": np.ascontiguousarray(pc0, dtype=f32),
        "pca": np.ascontiguousarray(pca, dtype=f32),
        "pc2": np.ascontiguousarray(pc2, dtype=f32),
    }


def get_program(native_silu=True, use_f32r=True):
    key = ("nc", native_silu, use_f32r)
    if key not in _CACHE:
        _CACHE[key] = _build_program(native_silu, use_f32r)
    return _CACHE[key]


def make_in_maps(x, params):
    import ml_dtypes

    B = x.shape[0]
    per = B // N_CORES
    xr = np.ascontiguousarray(x.reshape(B, CIN, N).astype(ml_dtypes.bfloat16))
    return [
        {"xs": xr[c * per:(c + 1) * per], **params}
        for c in range(N_CORES)
    ]


def kernel(x, w1, bn1_g, bn1_b, bn1_m, bn1_v, wq, bq, wk, bk, wv, bv,
           rel_h, rel_w, w2, bn2_g, bn2_b, bn2_m, bn2_v):
    from concourse.bass_utils import run_bass_kernel_spmd

    nc = get_program()
    params = _prepare_params(w1, bn1_g, bn1_b, bn1_m, bn1_v, wq, bq, wk, bk,
                             wv, bv, rel_h, rel_w, w2, bn2_g, bn2_b, bn2_m,
                             bn2_v)
    in_maps = make_in_maps(x, params)
    res = run_bass_kernel_spmd(nc, in_maps, core_ids=list(range(N_CORES)))
    out = np.concatenate([res.results[c]["ys"] for c in range(N_CORES)], axis=0)
    return np.ascontiguousarray(out.reshape(32, CIN, 32, 32), dtype=np.float32)
